# revision 1
# baseline (speedup 1.0000x reference)
"""Multi-head attention (sparse/causal+valid_len) Bass kernel for TRN2.

Sharding: 8 cores = 2 batches x 4 head-groups (4 heads each).
Each core: this batch's x-tensors + its head-group's weight slices,
computes a partial (S, D_MODEL) output (its heads' contribution through
w_o); host sums the 4 partials per batch and adds biases.

Layouts (bf16 compute, fp32 accumulate in PSUM):
  qT, kT  [head_dim(2 heads=128), S]  - transposed projections
  v       [k, 65*4]                   - natural, ones col per head (denom)
  scoresT [k, q] in PSUM -> exp on ACT (scale=1/8, bias=valid-mask) -> bf16
  causal zeroing of diagonal tiles via gpsimd affine_select
  attnV   [q, 65] accumulate over k-chunks; col 64 = softmax denominator
  normalize via DVE reciprocal + tensor_scalar_mul -> attn_out bf16
  PE-transpose attn_out -> attn_outT [hd, q] -> out-proj vs woT -> y fp32
"""

import numpy as np
import ml_dtypes

import concourse.bass as bass
import concourse.mybir as mybir
import concourse.tile as tile
from concourse.masks import make_identity

BF16 = mybir.dt.bfloat16
FP32 = mybir.dt.float32

S = 2048
D = 1024
HEADS_PER_CORE = 4   # head-group size
DH = 64
HD = HEADS_PER_CORE * DH          # 256
NEG = -1.0e5                      # additive mask; exp underflows to exactly 0

_MAX_WAITS = 1  # this container's walrus allows 1 sync wait per instruction


def fix_multi_waits(nc, max_waits: int = _MAX_WAITS):
    """Split >max_waits sem waits onto EventSemaphore insts placed just
    before the owning instruction (same engine => same semantics)."""
    import bass_rust
    n = 0
    for f in nc.m.functions:
        for bb in f.blocks:
            out = []
            changed = False
            for ins in bb.instructions:
                si = ins.sync_info
                waits = list(si.on_wait) if si is not None else []
                if len(waits) > max_waits:
                    changed = True
                    extra = waits[:-max_waits]
                    si.on_wait = waits[-max_waits:]
                    for i in range(0, len(extra), max_waits):
                        n += 1
                        es = mybir.InstEventSemaphore(
                            name=f"{ins.name}-esw{i}", ins=[], outs=[])
                        es.engine = ins.engine
                        es.sync_info = bass_rust.SyncInfo(
                            on_wait=extra[i:i + max_waits], on_update=[])
                        out.append(es)
                out.append(ins)
            if changed:
                bb.instructions = out
    return n


def build_kernel(KC: int, opts=None):
    opts = dict(opts or {})
    BIG_KC = KC > 8   # large valid_len: keep SBUF in budget
    EXP_BUFS = (2 * KC) if BIG_KC else (4 * KC + 2)
    PSCORE = opts.get("pscore", 2)
    PYO = opts.get("pyo", 2)
    PSMALL = opts.get("psmall", 2)
    ATTN_RATIO = opts.get("attn_ratio", 2)
    Y_ENG = opts.get("y_eng", "split")
    """Build the per-core Bass program. KC = number of 128-wide key chunks."""
    KP = KC * 128
    NQT = S // 128     # 16 query tiles of 128
    NQB = S // 512     # 4 query blocks of 512
    DM = D // 128      # 8 contraction chunks

    nc = bass.Bass()

    # DRAM I/O (per-core values supplied via in_maps)
    xqT_d = nc.dram_tensor("xqT", [D, S], BF16, kind="ExternalInput")
    xkT_d = nc.dram_tensor("xkT", [D, KP], BF16, kind="ExternalInput")
    xvT_d = nc.dram_tensor("xvT", [D, KP], BF16, kind="ExternalInput")
    wqT_d = nc.dram_tensor("wqT", [D, HD], BF16, kind="ExternalInput")
    wkT_d = nc.dram_tensor("wkT", [D, HD], BF16, kind="ExternalInput")
    wvT_d = nc.dram_tensor("wvT", [D, HD], BF16, kind="ExternalInput")
    woT_d = nc.dram_tensor("woT", [HD, D], BF16, kind="ExternalInput")
    vmask_d = nc.dram_tensor("vmask", [128, KC], FP32, kind="ExternalInput")
    bqk_d = nc.dram_tensor("bqk", [128, 4], FP32, kind="ExternalInput")
    y_d = nc.dram_tensor("y", [S, D], FP32, kind="ExternalOutput")

    with tile.TileContext(nc) as tc:
        with (
            tc.tile_pool(name="const", bufs=1) as cpool,
            tc.tile_pool(name="win", bufs=1) as wpool,
            tc.tile_pool(name="qkv", bufs=1) as qkvpool,
            tc.tile_pool(name="ao", bufs=4) as aopool,
            tc.tile_pool(name="ysb", bufs=3) as ypool,
            tc.tile_pool(name="ps_score", bufs=PSCORE, space="PSUM") as pscore,
            tc.tile_pool(name="ps_yo", bufs=PYO, space="PSUM") as pyo,
            tc.tile_pool(name="ps_small", bufs=PSMALL, space="PSUM") as psmall,
        ):
            from contextlib import ExitStack
            xstack = ExitStack()
            xpool = xstack.enter_context(tc.tile_pool(name="xin", bufs=1))
            estack = ExitStack()
            epool = None
            if not BIG_KC:
                epool = estack.enter_context(
                    tc.tile_pool(name="expp", bufs=EXP_BUFS))
            # ---- constants (tiny, needed early) ----
            ident = cpool.tile([128, 128], BF16, tag="ident")
            make_identity(nc, ident[:, :])
            # ---- loads, ordered so scores(qb0) unblocks ASAP:
            # wk+xk (kT proj), wq+xq (q proj), then v-path, wo last ----
            wkT = wpool.tile([128, DM, HD], BF16, tag="wkT")
            wk_r = wkT_d[:].rearrange("(c p) f -> p c f", p=128)
            nc.sync.dma_start(wkT[:, 0:2, :], wk_r[:, 0:2, :])
            xkT = xpool.tile([128, DM, KP], BF16, tag="xkT")
            xk_r = xkT_d[:].rearrange("(c p) f -> p c f", p=128)
            nc.sync.dma_start(xkT[:, 0:2, :], xk_r[:, 0:2, :])
            nc.sync.dma_start(wkT[:, 2:DM, :], wk_r[:, 2:DM, :])
            vmask = cpool.tile([128, KC], FP32, tag="vmask")
            nc.sync.dma_start(vmask[:, :], vmask_d[:, :])
            bqk = cpool.tile([128, 4], FP32, tag="bqk")
            nc.sync.dma_start(bqk[:, :], bqk_d[:, :])
            for c in range(2, DM):
                nc.sync.dma_start(xkT[:, c, :], xk_r[:, c, :])
            wqT = wpool.tile([128, DM, HD], BF16, tag="wqT")
            nc.sync.dma_start(
                wqT[:, :, :], wqT_d[:].rearrange("(c p) f -> p c f", p=128))
            # xq per query-block (1MB each): scores(qb0) unblocks after the
            # first block; v-path loads overlap attention of qb0
            xqT = xpool.tile([128, DM, S], BF16, tag="xqT")
            xq_r = xqT_d[:].rearrange("(c p) f -> p c f", p=128)
            nc.sync.dma_start(xqT[:, :, 0:512], xq_r[:, :, 0:512])
            wvT = wpool.tile([128, DM, HD], BF16, tag="wvT")
            nc.sync.dma_start(
                wvT[:, :, :], wvT_d[:].rearrange("(c p) f -> p c f", p=128))
            xvT = xpool.tile([128, DM, KP], BF16, tag="xvT")
            xv_r = xvT_d[:].rearrange("(c p) f -> p c f", p=128)
            for c in range(0, DM, 4):
                nc.sync.dma_start(xvT[:, c:c + 4, :], xv_r[:, c:c + 4, :])
            for qs in range(512, S, 512):
                nc.sync.dma_start(
                    xqT[:, :, qs:qs + 512], xq_r[:, :, qs:qs + 512])
            woT = wpool.tile([128, 2, D], BF16, tag="woT")
            nc.sync.dma_start(
                woT[:, :, :], woT_d[:].rearrange("(c p) f -> p c f", p=128))

            # ---- K projection (gates scores qb0 -> fully first) ----
            kT = [qkvpool.tile([128, KP], BF16, tag=f"kT{j}", name=f"kT{j}") for j in range(2)]
            for ks in range(0, KP, 512):
                for j in range(2):
                    w = min(512, KP - ks)
                    ps = pyo.tile([128, 512], FP32, tag="psy")
                    for c in range(DM):
                        nc.tensor.matmul(
                            ps[:, :w],
                            wkT[:, c, 128 * j:128 * j + 128],
                            xkT[:, c, ks:ks + w],
                            start=(c == 0), stop=(c == DM - 1))
                    nc.vector.tensor_scalar_add(
                        kT[j][:, ks:ks + w], ps[:, :w], bqk[:, 2 + j:3 + j])

            # ---- V projection: generator, interleaved during qb0 scores
            # (xv loads after xq block 0; v only needed by attnV of qb0) ----
            v_t = [qkvpool.tile([128, HEADS_PER_CORE * 65], BF16,
                                tag=f"v{kb}", name=f"v{kb}")
                   for kb in range(KC)]

            def emit_vproj():
                for kb in range(KC):
                    vt = v_t[kb]
                    nc.gpsimd.memset(vt[:, :], 1.0)  # ones cols = denom trick
                    ps = pyo.tile([128, 512], FP32, tag="psy",
                                  name=f"psv{kb}")
                    for c in range(DM):
                        nc.tensor.matmul(
                            ps[:, :HD],
                            xvT[:, c, 128 * kb:128 * kb + 128],
                            wvT[:, c, :],
                            start=(c == 0), stop=(c == DM - 1))
                    vt3 = vt[:].rearrange("p (h e) -> p h e", e=65)
                    nc.vector.tensor_copy(
                        vt3[:, :, 0:64],
                        ps[:, :HD].rearrange("p (h e) -> p h e", e=64))
                    yield

            # ---- Q projection (emitted per query block, pipelined) ----
            qT = [qkvpool.tile([128, S], BF16, tag=f"qT{j}", name=f"qT{j}") for j in range(2)]

            def emit_qproj(qb):
                qs = 512 * qb
                for j in range(2):
                    ps = pyo.tile([128, 512], FP32, tag="psy",
                                  name=f"psq{qb}_{j}")
                    for c in range(DM):
                        nc.tensor.matmul(
                            ps[:, :],
                            wqT[:, c, 128 * j:128 * j + 128],
                            xqT[:, c, qs:qs + 512],
                            start=(c == 0), stop=(c == DM - 1))
                    nc.vector.tensor_scalar_add(
                        qT[j][:, qs:qs + 512], ps[:, :], bqk[:, j:j + 1])
                    yield

            # ---- attention + output projection, per 512-query block ----
            # software-pipelined: scores/exp for qb+1 are emitted before
            # attnV/outproj of qb so PE never waits on ACT's exp pass
            attn_oT = [qkvpool.tile([128, S], BF16, tag=f"aoT{j}", name=f"aoT{j}")
                       for j in range(2)]
            exp_stage = {}
            att_tiles = {}
            epool_ref = [None]

            def emit_scores(qb, pair_major=False):
                # generator: yields after each (kt, pair) score unit
                ktm = min(4 * qb + 3, KC - 1)   # causal+valid key-chunk bound
                # scoresT [k, q] -> exp -> expT tiles (bf16)
                expT = [[None] * (ktm + 1) for _ in range(HEADS_PER_CORE)]
                exp_qlo = [0] * (ktm + 1)
                exp_stage[qb] = (expT, exp_qlo)
                kt_j = ([(kt, j) for j in range(2) for kt in range(ktm + 1)]
                        if pair_major else
                        [(kt, j) for kt in range(ktm + 1) for j in range(2)])
                for kt, j in kt_j:
                    # causal: queries below 128*kt never see this k chunk
                    qlo = max(0, 128 * kt - 512 * qb)
                    exp_qlo[kt] = qlo
                    w = 512 - qlo
                    if True:
                        # both row-halves (heads 2j, 2j+1) share one psum
                        # tile (different banks -> still concurrent on PE)
                        # and one exp + one causal-select instruction
                        ps = pscore.tile([128, 2, 512], FP32, tag="pssc",
                                         name=f"pssc{qb}_{kt}_{j}")
                        for r in range(2):
                            nc.tensor.matmul(
                                ps[:, r, :w],
                                kT[j][64 * r:64 * r + 64,
                                      128 * kt:128 * kt + 128],
                                qT[j][64 * r:64 * r + 64,
                                      512 * qb + qlo:512 * qb + 512],
                                start=True, stop=True)
                        et = epool_ref[0].tile([128, 2, w], BF16, tag="expT",
                                        name=f"expT{qb}_{kt}_{j}")
                        nc.scalar.activation(
                            et[:, :, :], ps[:, :, :w],
                            mybir.ActivationFunctionType.Exp,
                            bias=vmask[:, kt:kt + 1], scale=0.125)
                        if 128 * kt + 127 > 512 * qb + qlo:
                            # zero strictly-above-diagonal: keep q >= k
                            # (r-dim coefficient 0: same mask per head)
                            nc.gpsimd.affine_select(
                                out=et[:, :, :], in_=et[:, :, :],
                                compare_op=mybir.AluOpType.is_ge,
                                fill=0.0,
                                base=512 * qb + qlo - 128 * kt,
                                pattern=[[0, 2], [1, w]],
                                channel_multiplier=-1)
                        expT[2 * j][kt] = et
                        expT[2 * j + 1][kt] = et
                        yield



            def emit_attn(qb, pairs=(0, 1), pop=True):
                if pop:
                    expT, exp_qlo = exp_stage.pop(qb)
                else:
                    expT, exp_qlo = exp_stage[qb]
                for qq in range(4):             # 128-query tiles in this block
                    qt = 4 * qb + qq
                    att = att_tiles.setdefault(
                        qt, aopool.tile([128, HD], BF16, tag="att",
                                        name=f"att{qt}"))
                    for h in [2 * j + r for j in pairs for r in range(2)]:
                        ktm_q = min(qt, KC - 1)
                        po = psmall.tile([128, 65], FP32, tag="pso")
                        for kt in range(ktm_q + 1):
                            c0 = 128 * qq - exp_qlo[kt]
                            nc.tensor.matmul(
                                po[:, :],
                                expT[h][kt][:, h % 2, c0:c0 + 128],
                                v_t[kt][:, 65 * h:65 * h + 65],
                                start=(kt == 0), stop=(kt == ktm_q))
                        rec = aopool.tile([128, 1], FP32, tag="rec")
                        nc.vector.reciprocal(rec[:, :], po[:, 64:65])
                        nc.vector.tensor_scalar_mul(
                            att[:, DH * h:DH * h + DH], po[:, :64], rec[:, :])
                        yield
                    if pairs[-1] != 1:
                        continue
                    # transpose attn_out -> attn_oT (per head pair)
                    for j in range(2):
                        pst = psmall.tile([128, 128], BF16, tag="pso")
                        nc.tensor.transpose(
                            pst[:, :], att[:, 128 * j:128 * j + 128],
                            ident[:, :])
                        nc.vector.tensor_copy(
                            attn_oT[j][:, 128 * qt:128 * qt + 128], pst[:, :])

                    # output projection for this query tile
                    ys = ypool.tile([128, D], FP32, tag="ysb")
                    for n in range(2):
                        ps = pyo.tile([128, 512], FP32, tag="psy")
                        for hc in range(2):
                            nc.tensor.matmul(
                                ps[:, :],
                                attn_oT[hc][:, 128 * qt:128 * qt + 128],
                                woT[:, hc, 512 * n:512 * n + 512],
                                start=(hc == 0), stop=(hc == 1))
                        # split PSUM evacuation between ACT and DVE
                        if Y_ENG == "act" or (Y_ENG == "split" and n == 0):
                            nc.scalar.copy(ys[:, 512 * n:512 * n + 512],
                                           ps[:, :])
                        else:
                            nc.vector.tensor_copy(
                                ys[:, 512 * n:512 * n + 512], ps[:, :])
                    nc.sync.dma_start(
                        y_d[128 * qt:128 * qt + 128, :], ys[:, :])
                    yield

            def drain_gen(g, n=None):
                i = 0
                for _ in g:
                    i += 1
                    if n is not None and i >= n:
                        return False
                return True

            if epool is not None:
                epool_ref[0] = epool
            # qb0's projection up front; later projections interleave one
            # block ahead of their scores
            for _ in emit_qproj(0):
                pass
            vp = emit_vproj()
            if BIG_KC:
                # all projections upfront, then release x inputs from SBUF
                # and only then open the (large) exp pool in the freed zone
                for _ in vp:
                    pass
                for qb_ in range(1, NQB):
                    for _ in emit_qproj(qb_):
                        pass
                vp = None
                xstack.close()
                epool_ref[0] = estack.enter_context(
                    tc.tile_pool(name="expp", bufs=EXP_BUFS))
            for qb in range(NQB + 1):
                sc = emit_scores(qb) if qb < NQB else None
                at = (emit_attn(qb - 1)
                      if qb >= 1 and qb - 1 in exp_stage else None)
                qp = (emit_qproj(qb + 1)
                      if (not BIG_KC and qb + 1 < NQB) else None)
                done_sc = sc is None
                done_at = at is None
                done_qp = qp is None
                if qb == 0 and vp is not None:
                    at, done_at = vp, False
                    vp = None
                while not (done_sc and done_at and done_qp):
                    if not done_sc:
                        try:
                            next(sc)
                        except StopIteration:
                            done_sc = True
                    if not done_at:
                        for _ in range(ATTN_RATIO):
                            try:
                                next(at)
                            except StopIteration:
                                done_at = True
                                break
                    if not done_qp:
                        try:
                            next(qp)
                        except StopIteration:
                            done_qp = True

            estack.close()
            if not BIG_KC:
                xstack.close()

    fix_multi_waits(nc)
    return nc


def prepare_inputs(inputs):
    """Host-side shard/cast/transpose. Returns (in_maps, KC, host_bias)."""
    f32 = np.float32
    xq = np.asarray(inputs["will_be_queries"], f32)
    xk = np.asarray(inputs["will_be_keys"], f32)
    xv = np.asarray(inputs["will_be_values"], f32)
    L = np.asarray(inputs["valid_len"]).astype(np.int64)
    w_q = np.asarray(inputs["w_q"], f32)
    w_k = np.asarray(inputs["w_k"], f32)
    w_v = np.asarray(inputs["w_v"], f32)
    w_o = np.asarray(inputs["w_o"], f32)
    b_q = np.asarray(inputs["b_q"], f32)
    b_k = np.asarray(inputs["b_k"], f32)
    b_o = np.asarray(inputs["b_o"], f32)
    b_v = np.asarray(inputs["b_v"], f32)

    B = xq.shape[0]
    Lmax = int(L.max())
    KC = (Lmax + 127) // 128
    KP = KC * 128
    bf = ml_dtypes.bfloat16

    def t_bf(a):  # (r, c) -> transposed bf16 contiguous
        return np.ascontiguousarray(a.T).astype(bf)

    in_maps = []
    for core in range(8):
        b, hg = divmod(core, 4)
        rows = slice(HD * hg, HD * hg + HD)
        vm = np.full((128, KC), 0.0, f32)
        k_idx = (np.arange(KC)[None, :] * 128 + np.arange(128)[:, None])
        vm[k_idx >= L[b]] = NEG
        bqk = np.zeros((128, 4), f32)
        bqk[:, 0] = b_q[rows][:128]
        bqk[:, 1] = b_q[rows][128:]
        bqk[:, 2] = b_k[rows][:128]
        bqk[:, 3] = b_k[rows][128:]
        in_maps.append({
            "xqT": t_bf(xq[b]),
            "xkT": t_bf(xk[b][:KP]),
            "xvT": t_bf(xv[b][:KP]),
            "wqT": t_bf(w_q[rows]),
            "wkT": t_bf(w_k[rows]),
            "wvT": t_bf(w_v[rows]),
            "woT": t_bf(w_o[:, rows]),
            "vmask": vm,
            "bqk": bqk,
        })
    # exact host-side bias correction: y += b_o + w_o @ b_v
    host_bias = (b_o + w_o @ b_v).astype(f32)
    return in_maps, KC, host_bias


def combine_outputs(results, host_bias):
    B = 2
    out = np.zeros((B, S, D), np.float32)
    for core, res in enumerate(results):
        b = core // 4
        out[b] += res["y"]
    out += host_bias[None, None, :]
    return out


# ---------------------------------------------------------------------------
# Harness entry point: full (unsharded) inputs -> full output.
# Shards across the 8 NeuronCores as 2 batches x 4 head-groups, runs the
# Bass kernel SPMD, and reduces the per-core partial outputs on the host.
# ---------------------------------------------------------------------------
def kernel(**inputs) -> np.ndarray:
    from concourse.bass_utils import run_bass_kernel_spmd

    in_maps, KC, host_bias = prepare_inputs(inputs)
    nc = build_kernel(KC, {"attn_ratio": 4})
    res = run_bass_kernel_spmd(nc, in_maps, list(range(8)))
    return combine_outputs(res.results, host_bias)

